# revision 44
# baseline (speedup 1.0000x reference)
"""Trainium2 Bass kernel for causal multi-head attention (B=4, T=2048, D=1024, H=16).

Sharding: 8 cores = 4 batches x 2 head-groups (8 heads each).
Per core pipeline (Tile framework, single SPMD program):
  phase 1: Q/K projections into transposed per-head-pair layout QT/KT [128=2*64, 512]
           per t-range, V projection into [t, 8*65] layout (65th col = ones).
  phase 2: per (q-range of 512, head-pair): causal flash attention in transposed
           layout: ST[k,q] = KT^T @ QT (row-packed matmul pair), PT = exp(ST) (ACT),
           triangle mask on diagonal 128x128 sub-blocks (DVE), OT += [V|1]^T @ PT,
           normalize via fast reciprocal of the rowsum row.
  phase 3: output projection YT[dout, t] = Wo_gT^T @ OT + bias, one bf16
           ReduceScatter per t-range across the batch pair.
Projection work for range j+1 and output-projection work for range j-1 are
emitted interleaved into attention(j)'s loop so the tensor queue can fill the
ACT-paced gaps; per-range tiles keep the dependence graph free of false WARs.
Host: transpose/slice weights, assemble [B, T, D] from per-core [512, T] halves.
"""

import numpy as np

B, T, D, H, HD = 4, 2048, 1024, 16, 64
NCORES = 8
NP = 4          # head pairs per core
NJ = 4          # q-ranges of 512
QW = 512
TB = T // 128   # 16

_CACHE = {}


def _build_nc():
    import concourse.mybir as mybir
    import concourse.tile as tile
    from concourse import bacc

    F32 = mybir.dt.float32
    BF16 = mybir.dt.bfloat16
    AF = mybir.ActivationFunctionType
    ALU = mybir.AluOpType

    nc = bacc.Bacc(None, target_bir_lowering=False)
    # all inputs pre-tiled on the host so every load is a contiguous DMA
    xt_d = nc.declare_dram_parameter("xt", [NJ, 128, 8, QW], BF16, isOutput=False)
    wq_d = nc.declare_dram_parameter("wq", [128, 8, 512], BF16, isOutput=False)
    wk_d = nc.declare_dram_parameter("wk", [128, 8, 512], BF16, isOutput=False)
    wv_d = nc.declare_dram_parameter("wv", [128, 8, 512], BF16, isOutput=False)
    wo_d = nc.declare_dram_parameter("wo", [128, 4, D], BF16, isOutput=False)
    bias_d = nc.declare_dram_parameter("bias", [128, 8], F32, isOutput=False)
    mask_d = nc.declare_dram_parameter("mask", [128, 128], BF16, isOutput=False)
    yt_d = nc.declare_dram_parameter("yt", [NJ, 512, QW], BF16, isOutput=True)
    # (RS cannot scatter straight into the IO tensor: NEFF compile fails)

    RG = [[0, 1], [2, 3], [4, 5], [6, 7]]

    with tile.TileContext(nc) as tc:
        with (
            tc.tile_pool(name="persist", bufs=1) as pers,
            tc.tile_pool(name="work", bufs=1) as work,
            tc.tile_pool(name="dram", bufs=1, space="DRAM") as dram,
            tc.tile_pool(name="psum", bufs=1, space="PSUM") as psum,
        ):
            # per-q-range tiles (separate objects -> no false WAR between ranges)
            qt = [pers.tile([128, NP, QW], BF16, name=f"qt{_j}", tag=f"qt{_j}") for _j in range(NJ)]
            kt = [pers.tile([128, NP, QW], BF16, name=f"kt{_j}", tag=f"kt{_j}") for _j in range(NJ)]
            v = [pers.tile([128, 4, 8 * 65], BF16, name=f"v{_j}", tag=f"v{_j}") for _j in range(NJ)]
            ot = [pers.tile([128, NP, QW], BF16, name=f"ot{_j}", tag=f"ot{_j}") for _j in range(NJ)]
            m0 = pers.tile([128, 128], BF16)
            wo = pers.tile([128, 4, D], BF16)
            bias = pers.tile([128, 8], F32)
            wq = pers.tile([128, 8, 512], BF16)
            wk = pers.tile([128, 8, 512], BF16)
            wv = pers.tile([128, 8, 512], BF16)

            def load_xs(j):
                xs = work.tile([128, 8, QW], BF16, tag="xs", bufs=2)
                nc.sync.dma_start(out=xs[:], in_=xt_d[j])
                return xs

            xs0 = work.tile([128, 8, QW], BF16, tag="xs", bufs=2)
            nc.gpsimd.dma_start(out=wq[:, 0:4, :], in_=wq_d[:, 0:4, :])
            nc.sync.dma_start(out=xs0[:, 0:4, :], in_=xt_d[0, :, 0:4, :])
            nc.gpsimd.dma_start(out=wq[:, 4:8, :], in_=wq_d[:, 4:8, :])
            nc.sync.dma_start(out=xs0[:, 4:8, :], in_=xt_d[0, :, 4:8, :])
            xs_tiles = {0: xs0}
            nc.sync.dma_start(out=wk[:], in_=wk_d[:])
            nc.sync.dma_start(out=m0[:], in_=mask_d[:])
            nc.sync.dma_start(out=wv[:], in_=wv_d[:])
            nc.sync.dma_start(out=wo[:], in_=wo_d[:])
            nc.sync.dma_start(out=bias[:], in_=bias_d[:])

            yt_part = dram.tile([NJ, 1024, QW], BF16)
            yt_rs = dram.tile([NJ, 512, QW], BF16)


            def proj_qk_group(j, p, w_sb, dst):
                xs = xs_tiles[j]
                acc = psum.tile([128, QW], F32, tag="small", bufs=2)
                for c in range(8):
                    nc.tensor.matmul(
                        acc[:], w_sb[:, c, p * 128:(p + 1) * 128], xs[:, c, :],
                        start=(c == 0), stop=(c == 7),
                    )
                nc.scalar.copy(dst[j][:, p, :], acc[:])

            def proj_v_group(j, sub):
                xs = xs_tiles[j]
                acc = psum.tile([128, QW], F32, tag="small", bufs=2)
                for c in range(8):
                    nc.tensor.matmul(
                        acc[:], xs[:, c, sub * 128:(sub + 1) * 128], wv[:, c, :],
                        start=(c == 0), stop=(c == 7),
                    )
                vblk = v[j][:, sub, :].rearrange("p (h c) -> p h c", c=65)
                nc.scalar.copy(
                    vblk[:, :, 0:64], acc[:].rearrange("p (h c) -> p h c", c=64)
                )
                nc.gpsimd.memset(vblk[:, :, 64:65], 1.0)

            def proj_fillers(j):
                yield lambda: xs_tiles.__setitem__(j, load_xs(j))
                for p in range(NP):
                    for w_sb, dst in ((wq, qt), (wk, kt)):
                        yield lambda j=j, p=p, w_sb=w_sb, dst=dst: proj_qk_group(j, p, w_sb, dst)
                for sub in range(4):
                    yield lambda j=j, sub=sub: proj_v_group(j, sub)

            def y_block(j, n):
                ysb = work.tile([128, QW], BF16, tag="ysb", bufs=3)
                yps = psum.tile([128, QW], F32, tag="small", bufs=2)
                for c in range(4):
                    nc.tensor.matmul(
                        yps[:], wo[:, c, n * 128:(n + 1) * 128], ot[j][:, c, :],
                        start=(c == 0), stop=(c == 3),
                    )
                nc.scalar.add(ysb[:], yps[:], bias[:, n:n + 1])
                nc.sync.dma_start(out=yt_part[j, n * 128:(n + 1) * 128, :], in_=ysb[:])

            def y_finalize(j):
                # one RS per range: the ~7us fixed cost per collective dominates,
                # so a single big call beats chunked ones
                nc.gpsimd.collective_compute(
                    "ReduceScatter", ALU.add, replica_groups=RG,
                    ins=[yt_part[j].opt()],
                    outs=[yt_rs[j].opt()],
                )
                nc.sync.dma_start(out=yt_d[j], in_=yt_rs[j])

            def y_fillers(j):
                for n in range(8):
                    yield lambda j=j, n=n: y_block(j, n)
                yield lambda j=j: y_finalize(j)

            def attention(j):
                """phase 2 for range j, with filler groups for Y(j-1)/proj(j+1)."""
                pf = list(proj_fillers(j + 1)) if j + 1 < NJ else []
                yf = list(y_fillers(j - 1)) if j > 0 else []
                fillers = pf[:1]  # xs prefetch first
                pf = pf[1:]
                # round-robin the remaining proj groups with Y(j-1) chunks
                k = max(len(pf), len(yf))
                for idx in range(k):
                    if idx < len(pf):
                        fillers.append(pf[idx])
                    if idx < len(yf):
                        fillers.append(yf[idx])
                n_iters = NP * (4 * j + 4)
                # hold back the last two (projection) fillers so the PE stays
                # busy through the final normalization chain (HAM clock gate)
                cap = len(fillers) - 2 if j + 1 < NJ else len(fillers)
                emitted = 0
                it = 0
                for p in range(NP):
                    hA, hB = 2 * p, 2 * p + 1
                    o_A = psum.tile([65, QW], F32, tag="o", bufs=2)
                    o_B = psum.tile([65, QW], F32, tag="o", bufs=2)
                    nkb = 4 * j + 4

                    def pv(kb, pt, lo):
                        kj, kb_l = kb // 4, kb % 4
                        nc.tensor.matmul(
                            o_A[:, lo:QW],
                            v[kj][:, kb_l, hA * 65:(hA + 1) * 65],
                            pt[:, lo:QW],
                            start=(kb == 0), stop=(kb == nkb - 1),
                        )
                        nc.tensor.matmul(
                            o_B[:, lo:QW],
                            v[kj][:, kb_l, hB * 65:(hB + 1) * 65],
                            pt[:, QW + lo:2 * QW],
                            start=(kb == 0), stop=(kb == nkb - 1),
                        )

                    pend = None
                    for kb in range(nkb):
                        o = kb - 4 * j  # diagonal offset; < 0 means full block
                        lo = 128 * o if o > 0 else 0
                        st = psum.tile([128, 1024], F32, tag="st", bufs=2)
                        kj, kb_l = kb // 4, kb % 4
                        kcols = slice(kb_l * 128, (kb_l + 1) * 128)
                        qcols = slice(lo, QW)
                        nc.tensor.matmul(
                            st[:, lo:QW],
                            kt[kj][0:64, p, kcols],
                            qt[j][0:64, p, qcols],
                            start=True, stop=True, tile_position=(0, 0),
                        )
                        nc.tensor.matmul(
                            st[:, QW + lo:2 * QW],
                            kt[kj][64:128, p, kcols],
                            qt[j][64:128, p, qcols],
                            start=True, stop=True, tile_position=(64, 0),
                        )
                        # PV for the previous block goes behind this block's
                        # scores in the tensor queue: by then its exp is done,
                        # so the PE never head-of-line blocks on the ACT
                        if pend is not None:
                            pend()
                        pt = work.tile([128, 1024], BF16, tag="pt", bufs=4)
                        nc.scalar.activation(
                            pt[:].rearrange("p (h q) -> p h q", h=2)[:, :, lo:QW],
                            st[:].rearrange("p (h q) -> p h q", h=2)[:, :, lo:QW],
                            AF.Exp,
                        )
                        if o >= 0:
                            nc.vector.tensor_mul(
                                pt[:, lo:lo + 128], pt[:, lo:lo + 128], m0[:]
                            )
                            nc.vector.tensor_mul(
                                pt[:, QW + lo:QW + lo + 128],
                                pt[:, QW + lo:QW + lo + 128],
                                m0[:],
                            )
                        pend = lambda kb=kb, pt=pt, lo=lo: pv(kb, pt, lo)
                        it += 1
                        while fillers and emitted < min(cap, (it * len(fillers)) // n_iters):
                            fillers[emitted]()
                            emitted += 1
                    pend()
                    # stage o out of PSUM quickly, then normalize from SBUF
                    ocp = work.tile([65, 1024], F32, tag="ocp", bufs=3)
                    nc.vector.tensor_copy(ocp[:, 0:QW], o_A[:])
                    nc.vector.tensor_copy(ocp[:, QW:1024], o_B[:])
                    rsum = work.tile([1, 1024], F32, tag="rsum", bufs=2)
                    nc.vector.tensor_copy(rsum[:], ocp[64:65, :])
                    rec = work.tile([1, 1024], F32, tag="rec", bufs=2)
                    bc = work.tile([64, 1024], F32, tag="bc", bufs=2)
                    # per-head halves: head A's broadcast overlaps head B's recip
                    nc.vector.reciprocal_approx_fast(rec[:, 0:QW], rsum[:, 0:QW])
                    nc.gpsimd.partition_broadcast(bc[:, 0:QW], rec[:, 0:QW], channels=64)
                    nc.vector.reciprocal_approx_fast(rec[:, QW:1024], rsum[:, QW:1024])
                    nc.gpsimd.partition_broadcast(bc[:, QW:1024], rec[:, QW:1024], channels=64)
                    nc.vector.tensor_mul(ot[j][0:64, p, :], ocp[0:64, 0:QW], bc[:, 0:QW])
                    nc.vector.tensor_mul(ot[j][64:128, p, :], ocp[0:64, QW:1024], bc[:, QW:1024])
                for f in fillers[emitted:]:
                    f()

            # phase 1 for j=0 up front, then attention(j) with interleaved fillers
            for p in range(NP):
                for w_sb, dst in ((wq, qt), (wk, kt)):
                    proj_qk_group(0, p, w_sb, dst)
            for sub in range(4):
                proj_v_group(0, sub)

            for j in range(NJ):
                attention(j)
            for n in range(8):
                y_block(NJ - 1, n)
            y_finalize(NJ - 1)

    nc.finalize()
    return nc


def _prep_inputs(x, Wq, Wk, Wv, Wo, bo):
    """Build the 8 per-core input maps (host-side layout prep only)."""
    import ml_dtypes

    scale = 1.0 / np.sqrt(np.float32(HD))
    kr = np.arange(128, dtype=np.float32)[:, None]
    qc = np.arange(128, dtype=np.float32)[None, :]
    m0 = (qc >= kr).astype(ml_dtypes.bfloat16)

    def tile_w(wt, nchunk):  # [D_in, N] -> [128, nchunk, N] contiguous
        return np.ascontiguousarray(
            wt.reshape(nchunk, 128, wt.shape[1]).transpose(1, 0, 2)
        )

    in_maps = []
    for c in range(NCORES):
        b, g = c // 2, c % 2
        hs = slice(g * 8, (g + 1) * 8)
        xtT = x[b].T.astype(ml_dtypes.bfloat16)  # [D, T]
        xt = np.ascontiguousarray(
            xtT.reshape(8, 128, NJ, QW).transpose(2, 1, 0, 3)
        )  # [NJ, 128, 8, QW]
        wq = tile_w((Wq[hs].reshape(512, D).T * scale).astype(ml_dtypes.bfloat16), 8)
        wk = tile_w(Wk[hs].reshape(512, D).T.astype(ml_dtypes.bfloat16), 8)
        wv = tile_w(Wv[hs].reshape(512, D).T.astype(ml_dtypes.bfloat16), 8)
        wo = tile_w(Wo[:, g * 512:(g + 1) * 512].T.astype(ml_dtypes.bfloat16), 4)
        if g == 0:
            bias = np.ascontiguousarray(bo.reshape(8, 128).T)
        else:
            bias = np.zeros((128, 8), np.float32)
        in_maps.append(
            {"xt": xt, "wq": wq, "wk": wk, "wv": wv, "wo": wo, "bias": bias, "mask": m0}
        )
    return in_maps


def _run(inputs, trace=False, trace_cores=None):
    from concourse.bass_utils import run_bass_kernel_spmd

    if "nc" not in _CACHE:
        _CACHE["nc"] = _build_nc()
    nc = _CACHE["nc"]
    in_maps = _prep_inputs(
        inputs["x"], inputs["Wq"], inputs["Wk"], inputs["Wv"], inputs["Wo"], inputs["bo"]
    )
    r = run_bass_kernel_spmd(
        nc, in_maps, list(range(NCORES)), trace=trace, trace_cores=trace_cores
    )
    y = np.empty((B, T, D), np.float32)
    for b in range(B):
        yt = np.concatenate(
            [
                r.results[2 * b]["yt"].transpose(1, 0, 2).reshape(512, T),
                r.results[2 * b + 1]["yt"].transpose(1, 0, 2).reshape(512, T),
            ],
            axis=0,
        )
        y[b] = yt.T.astype(np.float32)
    return y, r


def kernel(**inputs):
    y, _ = _run(inputs, trace=False)
    return y


# revision 45
# speedup vs baseline: 1.0174x; 1.0174x over previous
"""Trainium2 Bass kernel for causal multi-head attention (B=4, T=2048, D=1024, H=16).

Sharding: 8 cores = 4 batches x 2 head-groups (8 heads each).
Per core pipeline (Tile framework, single SPMD program):
  phase 1: Q/K projections into transposed per-head-pair layout QT/KT [128=2*64, 512]
           per t-range, V projection into [t, 8*65] layout (65th col = ones).
  phase 2: per (q-range of 512, head-pair): causal flash attention in transposed
           layout: ST[k,q] = KT^T @ QT (row-packed matmul pair), PT = exp(ST) (ACT),
           triangle mask on diagonal 128x128 sub-blocks (DVE), OT += [V|1]^T @ PT,
           normalize via fast reciprocal of the rowsum row.
  phase 3: output projection YT[dout, t] = Wo_gT^T @ OT + bias, one bf16
           ReduceScatter per t-range across the batch pair.
Projection work for range j+1 and output-projection work for range j-1 are
emitted interleaved into attention(j)'s loop so the tensor queue can fill the
ACT-paced gaps; per-range tiles keep the dependence graph free of false WARs.
Host: transpose/slice weights, assemble [B, T, D] from per-core [512, T] halves.
"""

import numpy as np

B, T, D, H, HD = 4, 2048, 1024, 16, 64
NCORES = 8
NP = 4          # head pairs per core
NJ = 4          # q-ranges of 512
QW = 512
TB = T // 128   # 16

_CACHE = {}


def _build_nc():
    import concourse.mybir as mybir
    import concourse.tile as tile
    from concourse import bacc

    F32 = mybir.dt.float32
    BF16 = mybir.dt.bfloat16
    AF = mybir.ActivationFunctionType
    ALU = mybir.AluOpType

    nc = bacc.Bacc(None, target_bir_lowering=False)
    # all inputs pre-tiled on the host so every load is a contiguous DMA
    xt_d = nc.declare_dram_parameter("xt", [NJ, 128, 8, QW], BF16, isOutput=False)
    wq_d = nc.declare_dram_parameter("wq", [128, 8, 512], BF16, isOutput=False)
    wk_d = nc.declare_dram_parameter("wk", [128, 8, 512], BF16, isOutput=False)
    wv_d = nc.declare_dram_parameter("wv", [128, 8, 512], BF16, isOutput=False)
    wo_d = nc.declare_dram_parameter("wo", [128, 4, D], BF16, isOutput=False)
    bias_d = nc.declare_dram_parameter("bias", [128, 8], F32, isOutput=False)
    mask_d = nc.declare_dram_parameter("mask", [128, 128], BF16, isOutput=False)
    yt_d = nc.declare_dram_parameter("yt", [NJ, 512, QW], BF16, isOutput=True)
    # (RS cannot scatter straight into the IO tensor: NEFF compile fails)

    RG = [[0, 1], [2, 3], [4, 5], [6, 7]]

    with tile.TileContext(nc) as tc:
        with (
            tc.tile_pool(name="persist", bufs=1) as pers,
            tc.tile_pool(name="work", bufs=1) as work,
            tc.tile_pool(name="dram", bufs=1, space="DRAM") as dram,
            tc.tile_pool(name="psum", bufs=1, space="PSUM") as psum,
        ):
            # per-q-range tiles (separate objects -> no false WAR between ranges)
            qt = [pers.tile([128, NP, QW], BF16, name=f"qt{_j}", tag=f"qt{_j}") for _j in range(NJ)]
            kt = [pers.tile([128, NP, QW], BF16, name=f"kt{_j}", tag=f"kt{_j}") for _j in range(NJ)]
            v = [pers.tile([128, 4, 8 * 65], BF16, name=f"v{_j}", tag=f"v{_j}") for _j in range(NJ)]
            ot = [pers.tile([128, NP, QW], BF16, name=f"ot{_j}", tag=f"ot{_j}") for _j in range(NJ)]
            m0 = pers.tile([128, 128], BF16)
            wo = pers.tile([128, 4, D], BF16)
            bias = pers.tile([128, 8], F32)
            wq = pers.tile([128, 8, 512], BF16)
            wk = pers.tile([128, 8, 512], BF16)
            wv = pers.tile([128, 8, 512], BF16)

            def load_xs(j):
                xs = work.tile([128, 8, QW], BF16, tag="xs", bufs=2)
                nc.sync.dma_start(out=xs[:], in_=xt_d[j])
                return xs

            xs0 = work.tile([128, 8, QW], BF16, tag="xs", bufs=2)
            nc.gpsimd.dma_start(out=wq[:, 0:4, :], in_=wq_d[:, 0:4, :])
            nc.sync.dma_start(out=xs0[:, 0:4, :], in_=xt_d[0, :, 0:4, :])
            nc.gpsimd.dma_start(out=wq[:, 4:8, :], in_=wq_d[:, 4:8, :])
            nc.sync.dma_start(out=xs0[:, 4:8, :], in_=xt_d[0, :, 4:8, :])
            xs_tiles = {0: xs0}
            nc.sync.dma_start(out=wk[:], in_=wk_d[:])
            nc.sync.dma_start(out=m0[:], in_=mask_d[:])
            nc.sync.dma_start(out=wv[:], in_=wv_d[:])
            nc.sync.dma_start(out=wo[:], in_=wo_d[:])
            nc.sync.dma_start(out=bias[:], in_=bias_d[:])

            yt_part = dram.tile([NJ, 1024, QW], BF16)
            yt_rs = dram.tile([NJ, 512, QW], BF16)


            def proj_qk_group(j, p, w_sb, dst):
                xs = xs_tiles[j]
                acc = psum.tile([128, QW], F32, tag="small", bufs=2)
                for c in range(8):
                    nc.tensor.matmul(
                        acc[:], w_sb[:, c, p * 128:(p + 1) * 128], xs[:, c, :],
                        start=(c == 0), stop=(c == 7),
                    )
                nc.vector.tensor_copy(dst[j][:, p, :], acc[:])

            def proj_v_group(j, sub):
                xs = xs_tiles[j]
                acc = psum.tile([128, QW], F32, tag="small", bufs=2)
                for c in range(8):
                    nc.tensor.matmul(
                        acc[:], xs[:, c, sub * 128:(sub + 1) * 128], wv[:, c, :],
                        start=(c == 0), stop=(c == 7),
                    )
                vblk = v[j][:, sub, :].rearrange("p (h c) -> p h c", c=65)
                nc.vector.tensor_copy(
                    vblk[:, :, 0:64], acc[:].rearrange("p (h c) -> p h c", c=64)
                )
                nc.gpsimd.memset(vblk[:, :, 64:65], 1.0)

            def proj_fillers(j):
                yield lambda: xs_tiles.__setitem__(j, load_xs(j))
                for p in range(NP):
                    for w_sb, dst in ((wq, qt), (wk, kt)):
                        yield lambda j=j, p=p, w_sb=w_sb, dst=dst: proj_qk_group(j, p, w_sb, dst)
                for sub in range(4):
                    yield lambda j=j, sub=sub: proj_v_group(j, sub)

            def y_block(j, n):
                ysb = work.tile([128, QW], BF16, tag="ysb", bufs=3)
                yps = psum.tile([128, QW], F32, tag="small", bufs=2)
                for c in range(4):
                    nc.tensor.matmul(
                        yps[:], wo[:, c, n * 128:(n + 1) * 128], ot[j][:, c, :],
                        start=(c == 0), stop=(c == 3),
                    )
                nc.vector.tensor_scalar_add(ysb[:], yps[:], bias[:, n:n + 1])
                nc.sync.dma_start(out=yt_part[j, n * 128:(n + 1) * 128, :], in_=ysb[:])

            def y_finalize(j):
                # one RS per range: the ~7us fixed cost per collective dominates,
                # so a single big call beats chunked ones
                nc.gpsimd.collective_compute(
                    "ReduceScatter", ALU.add, replica_groups=RG,
                    ins=[yt_part[j].opt()],
                    outs=[yt_rs[j].opt()],
                )
                nc.sync.dma_start(out=yt_d[j], in_=yt_rs[j])

            def y_fillers(j):
                for n in range(8):
                    yield lambda j=j, n=n: y_block(j, n)
                yield lambda j=j: y_finalize(j)

            def attention(j):
                """phase 2 for range j, with filler groups for Y(j-1)/proj(j+1)."""
                pf = list(proj_fillers(j + 1)) if j + 1 < NJ else []
                yf = list(y_fillers(j - 1)) if j > 0 else []
                fillers = pf[:1]  # xs prefetch first
                pf = pf[1:]
                # round-robin the remaining proj groups with Y(j-1) chunks
                k = max(len(pf), len(yf))
                for idx in range(k):
                    if idx < len(pf):
                        fillers.append(pf[idx])
                    if idx < len(yf):
                        fillers.append(yf[idx])
                n_iters = NP * (4 * j + 4)
                # hold back the last two (projection) fillers so the PE stays
                # busy through the final normalization chain (HAM clock gate)
                cap = len(fillers) - 2 if j + 1 < NJ else len(fillers)
                emitted = 0
                it = 0
                for p in range(NP):
                    hA, hB = 2 * p, 2 * p + 1
                    o_A = psum.tile([65, QW], F32, tag="o", bufs=2)
                    o_B = psum.tile([65, QW], F32, tag="o", bufs=2)
                    nkb = 4 * j + 4

                    def pv(kb, pt, lo):
                        kj, kb_l = kb // 4, kb % 4
                        nc.tensor.matmul(
                            o_A[:, lo:QW],
                            v[kj][:, kb_l, hA * 65:(hA + 1) * 65],
                            pt[:, lo:QW],
                            start=(kb == 0), stop=(kb == nkb - 1),
                        )
                        nc.tensor.matmul(
                            o_B[:, lo:QW],
                            v[kj][:, kb_l, hB * 65:(hB + 1) * 65],
                            pt[:, QW + lo:2 * QW],
                            start=(kb == 0), stop=(kb == nkb - 1),
                        )

                    pend = None
                    for kb in range(nkb):
                        o = kb - 4 * j  # diagonal offset; < 0 means full block
                        lo = 128 * o if o > 0 else 0
                        st = psum.tile([128, 1024], F32, tag="st", bufs=2)
                        kj, kb_l = kb // 4, kb % 4
                        kcols = slice(kb_l * 128, (kb_l + 1) * 128)
                        qcols = slice(lo, QW)
                        nc.tensor.matmul(
                            st[:, lo:QW],
                            kt[kj][0:64, p, kcols],
                            qt[j][0:64, p, qcols],
                            start=True, stop=True, tile_position=(0, 0),
                        )
                        nc.tensor.matmul(
                            st[:, QW + lo:2 * QW],
                            kt[kj][64:128, p, kcols],
                            qt[j][64:128, p, qcols],
                            start=True, stop=True, tile_position=(64, 0),
                        )
                        # PV for the previous block goes behind this block's
                        # scores in the tensor queue: by then its exp is done,
                        # so the PE never head-of-line blocks on the ACT
                        if pend is not None:
                            pend()
                        pt = work.tile([128, 1024], BF16, tag="pt", bufs=4)
                        nc.scalar.activation(
                            pt[:].rearrange("p (h q) -> p h q", h=2)[:, :, lo:QW],
                            st[:].rearrange("p (h q) -> p h q", h=2)[:, :, lo:QW],
                            AF.Exp,
                        )
                        if o >= 0:
                            nc.vector.tensor_mul(
                                pt[:, lo:lo + 128], pt[:, lo:lo + 128], m0[:]
                            )
                            nc.vector.tensor_mul(
                                pt[:, QW + lo:QW + lo + 128],
                                pt[:, QW + lo:QW + lo + 128],
                                m0[:],
                            )
                        pend = lambda kb=kb, pt=pt, lo=lo: pv(kb, pt, lo)
                        it += 1
                        while fillers and emitted < min(cap, (it * len(fillers)) // n_iters):
                            fillers[emitted]()
                            emitted += 1
                    pend()
                    # stage o out of PSUM quickly, then normalize from SBUF
                    ocp = work.tile([65, 1024], F32, tag="ocp", bufs=3)
                    nc.vector.tensor_copy(ocp[:, 0:QW], o_A[:])
                    nc.vector.tensor_copy(ocp[:, QW:1024], o_B[:])
                    rsum = work.tile([1, 1024], F32, tag="rsum", bufs=2)
                    nc.vector.tensor_copy(rsum[:], ocp[64:65, :])
                    rec = work.tile([1, 1024], F32, tag="rec", bufs=2)
                    bc = work.tile([64, 1024], F32, tag="bc", bufs=2)
                    # per-head halves: head A's broadcast overlaps head B's recip
                    nc.vector.reciprocal_approx_fast(rec[:, 0:QW], rsum[:, 0:QW])
                    nc.gpsimd.partition_broadcast(bc[:, 0:QW], rec[:, 0:QW], channels=64)
                    nc.vector.reciprocal_approx_fast(rec[:, QW:1024], rsum[:, QW:1024])
                    nc.gpsimd.partition_broadcast(bc[:, QW:1024], rec[:, QW:1024], channels=64)
                    nc.vector.tensor_mul(ot[j][0:64, p, :], ocp[0:64, 0:QW], bc[:, 0:QW])
                    nc.vector.tensor_mul(ot[j][64:128, p, :], ocp[0:64, QW:1024], bc[:, QW:1024])
                for f in fillers[emitted:]:
                    f()

            # phase 1 for j=0 up front, then attention(j) with interleaved fillers
            for p in range(NP):
                for w_sb, dst in ((wq, qt), (wk, kt)):
                    proj_qk_group(0, p, w_sb, dst)
            for sub in range(4):
                proj_v_group(0, sub)

            for j in range(NJ):
                attention(j)
            for n in range(8):
                y_block(NJ - 1, n)
            y_finalize(NJ - 1)

    nc.finalize()
    return nc


def _prep_inputs(x, Wq, Wk, Wv, Wo, bo):
    """Build the 8 per-core input maps (host-side layout prep only)."""
    import ml_dtypes

    scale = 1.0 / np.sqrt(np.float32(HD))
    kr = np.arange(128, dtype=np.float32)[:, None]
    qc = np.arange(128, dtype=np.float32)[None, :]
    m0 = (qc >= kr).astype(ml_dtypes.bfloat16)

    def tile_w(wt, nchunk):  # [D_in, N] -> [128, nchunk, N] contiguous
        return np.ascontiguousarray(
            wt.reshape(nchunk, 128, wt.shape[1]).transpose(1, 0, 2)
        )

    in_maps = []
    for c in range(NCORES):
        b, g = c // 2, c % 2
        hs = slice(g * 8, (g + 1) * 8)
        xtT = x[b].T.astype(ml_dtypes.bfloat16)  # [D, T]
        xt = np.ascontiguousarray(
            xtT.reshape(8, 128, NJ, QW).transpose(2, 1, 0, 3)
        )  # [NJ, 128, 8, QW]
        wq = tile_w((Wq[hs].reshape(512, D).T * scale).astype(ml_dtypes.bfloat16), 8)
        wk = tile_w(Wk[hs].reshape(512, D).T.astype(ml_dtypes.bfloat16), 8)
        wv = tile_w(Wv[hs].reshape(512, D).T.astype(ml_dtypes.bfloat16), 8)
        wo = tile_w(Wo[:, g * 512:(g + 1) * 512].T.astype(ml_dtypes.bfloat16), 4)
        if g == 0:
            bias = np.ascontiguousarray(bo.reshape(8, 128).T)
        else:
            bias = np.zeros((128, 8), np.float32)
        in_maps.append(
            {"xt": xt, "wq": wq, "wk": wk, "wv": wv, "wo": wo, "bias": bias, "mask": m0}
        )
    return in_maps


def _run(inputs, trace=False, trace_cores=None):
    from concourse.bass_utils import run_bass_kernel_spmd

    if "nc" not in _CACHE:
        _CACHE["nc"] = _build_nc()
    nc = _CACHE["nc"]
    in_maps = _prep_inputs(
        inputs["x"], inputs["Wq"], inputs["Wk"], inputs["Wv"], inputs["Wo"], inputs["bo"]
    )
    r = run_bass_kernel_spmd(
        nc, in_maps, list(range(NCORES)), trace=trace, trace_cores=trace_cores
    )
    y = np.empty((B, T, D), np.float32)
    for b in range(B):
        yt = np.concatenate(
            [
                r.results[2 * b]["yt"].transpose(1, 0, 2).reshape(512, T),
                r.results[2 * b + 1]["yt"].transpose(1, 0, 2).reshape(512, T),
            ],
            axis=0,
        )
        y[b] = yt.T.astype(np.float32)
    return y, r


def kernel(**inputs):
    y, _ = _run(inputs, trace=False)
    return y
